# revision 11
# baseline (speedup 1.0000x reference)
"""LSTM encoder (B=64, S=512, E=H=1024) on 8 trn2 NeuronCores.

Strategy:
  - Tensor-parallel over the 4H gate dimension: each core owns 128 hidden
    channels (x4 gates = 512 gate rows), the full batch (64), and the full
    sequence.
  - Phase 1 (parallel): embedding gather via dma_gather(transpose=True)
    directly into X^T layout, then gx = W_ih_local @ X^T for all 32768
    tokens, stored to DRAM as bf16.
  - Phase 2 (recurrence): 512 sequential steps. Per step: gates.T =
    identity-matmul(gx_t) + sum_k W_hh_chunk @ h_chunk (PSUM accumulate,
    one shared 256-col bank region per parity), sigmoid/tanh on ScalarE,
    cell update on VectorE. The new local h slice (128 ch x 64 batch,
    bf16) goes out as ONE remote_dma_broadcast to all 8 cores (self
    included) with a register-valued destination offset: every sender
    writes its own physical-slot column range on every receiver, so the
    SPMD program stays identical across cores and the per-step SWDGE
    desc-gen cost is one prep + one trigger instead of seven.
  - h lives in [-1,1]: bf16 exchange; c stays fp32 on-core.

Self-contained: hardcodes all shapes; host-side prep is numpy only.
"""

import os
import sys

sys.path.insert(0, "/opt/trn_rl_repo")

import numpy as np
import ml_dtypes

import concourse.bass as bass
import concourse.bacc as bacc
import concourse.mybir as mybir
from concourse.bass import ds

BF16 = ml_dtypes.bfloat16
AF = mybir.ActivationFunctionType
dt = mybir.dt

# problem constants
VOCAB, EMB, HID = 32000, 1024, 1024
B = 64
S = 512
CORES = 8
KC = 8            # contraction chunks of 128
NCHUNK = 4        # gate chunks per core (order: g, i, f, o)
G = NCHUNK * 128  # 512 gate rows per core
NT = 512          # tokens per phase-1 tile
TPT = NT // B     # timesteps per phase-1 tile (8)
# pytorch gate blocks in W rows: i, f, g, o ; our chunk order: g, i, f, o
CHUNK_TO_BLOCK = [2, 0, 1, 3]

# Logical(replica) -> physical TPB mapping on trn2 (driver V0 table, the
# per-chip base offset cancels): upper-die pairs are swapped. Broadcast
# slot q on every receiver holds the h slice of logical core _M[q]
# (each sender writes its own physical slot everywhere). _M is an
# involution, so _M[_M[q]] = q. HW-verified by a probe kernel.
_M = [0, 1, 2, 3, 6, 7, 4, 5]


def build(nc_steps=S, exchange="remote", nbcast=7, wait_rsem=True, fp8=0):
    """Emit the SPMD bass program (identical on all 8 cores).

    fp8=0: bf16 everywhere. fp8=1: phase-1 W_ih/x in fp8e4 (DoubleRow).
    fp8=2: also phase-2 W_hh/h in fp8e4. Weights are host-scaled by 64;
    activations rescale by 1/64.
    """
    nsteps = nc_steps
    TT = B * nsteps // NT  # number of phase-1 token tiles
    assert B * nsteps % NT == 0
    DR = mybir.MatmulPerfMode.DoubleRow
    x_dt = dt.bfloat16
    h_dt = dt.float8e4 if fp8 >= 2 else dt.bfloat16

    nc = bacc.Bacc(None, target_bir_lowering=False)

    # ---- kernel I/O (per core) ----
    emb_d = nc.declare_dram_parameter("emb16", [VOCAB, EMB], x_dt, isOutput=False)
    idx_d = nc.declare_dram_parameter("idx", [TT, 128, NT // 16], dt.int16, isOutput=False)
    wih_d = nc.declare_dram_parameter("w_ih", [128, KC * G], dt.bfloat16, isOutput=False)
    whh_d = nc.declare_dram_parameter("w_hh", [128, KC * G], h_dt, isOutput=False)
    ident_d = nc.declare_dram_parameter("ident", [128, 128], dt.bfloat16, isOutput=False)
    gbias_d = nc.declare_dram_parameter("gbias", [128, NCHUNK], dt.float32, isOutput=False)
    slot_d = nc.declare_dram_parameter("slot", [1, 1], dt.int32, isOutput=False)
    out_d = nc.declare_dram_parameter("out", [2, 128, B], dt.float32, isOutput=True)

    # ---- DRAM scratch ----
    gx_d = nc.dram_tensor("gx", [128, nsteps, NCHUNK * B], dt.bfloat16)
    bar_in = nc.dram_tensor("bar_in", [128, 4], dt.float32)
    bar_out = nc.dram_tensor("bar_out", [128, 4], dt.float32, addr_space="Shared")

    # ---- semaphores ----
    cc_sem = nc.alloc_semaphore("cc_sem")
    bar_sem = nc.alloc_semaphore("bar_sem")
    bardma_sem = nc.alloc_semaphore("bardma_sem")
    wload = nc.alloc_semaphore("wload")
    g_sem = [nc.alloc_semaphore("g_sem0"), nc.alloc_semaphore("g_sem1")]
    mm1 = nc.alloc_semaphore("mm1")
    cp_sem = nc.alloc_semaphore("cp_sem")
    st_sem = [nc.alloc_semaphore("st_sem0"), nc.alloc_semaphore("st_sem1")]
    gxd = [nc.alloc_semaphore("gxd0"), nc.alloc_semaphore("gxd1")]
    idm = nc.alloc_semaphore("idm")
    mmr = nc.alloc_semaphore("mmr")
    act_s = nc.alloc_semaphore("act_s")
    dve_s = nc.alloc_semaphore("dve_s")
    prep_s = nc.alloc_semaphore("prep_s")
    # parity-split: exchange for h(t) uses index (t+1)%2; the 2-step
    # pipeline separation keeps parity chains disjoint.
    rsem = [nc.alloc_semaphore("rsem0"), nc.alloc_semaphore("rsem1")]  # +16 per exchange
    lsem = [nc.alloc_semaphore("lsem0"), nc.alloc_semaphore("lsem1")]  # +16 per exchange
    fin = nc.alloc_semaphore("fin")

    from contextlib import ExitStack

    with ExitStack() as ctx:
        sb = lambda name, shape, d: ctx.enter_context(nc.sbuf_tensor(name, shape, d))
        idx_sb = sb("idx_sb", [128, TT * (NT // 16)], dt.int16)
        wih_sb = sb("wih_sb", [128, KC * G], dt.bfloat16)
        whh_sb = sb("whh_sb", [128, KC * G], h_dt)
        ident_sb = sb("ident_sb", [128, 128], dt.bfloat16)
        gbias_sb = sb("gbias_sb", [128, NCHUNK], dt.float32)
        slot_sb = sb("slot_sb", [1, 1], dt.int32)
        xt = [sb(f"xt{i}", [128, KC, NT], x_dt) for i in range(2)]
        stage = [sb(f"stage{i}", [128, TPT * NCHUNK * B], dt.bfloat16) for i in range(2)]
        hg = [sb(f"hg{i}", [128, CORES * B], h_dt) for i in range(2)]
        hmine = [sb(f"hmine{i}", [128, B], h_dt) for i in range(2)]
        gxt = [sb(f"gxt{i}", [128, NCHUNK * B], dt.bfloat16) for i in range(2)]
        sg = sb("sg", [128, NCHUNK * B], dt.float32)
        ig_sb = sb("ig_sb", [128, B], dt.float32)
        fc_sb = sb("fc_sb", [128, B], dt.float32)
        thc_sb = sb("thc_sb", [128, B], dt.float32)
        c_sb = sb("c_sb", [128, B], dt.float32)
        hout_sb = sb("hout_sb", [128, B], dt.float32)
        bar_sb = sb("bar_sb", [128, 4], dt.float32)
        # PSUM: 8 tensors of [128, 512] fp32 = 8 full banks. Phase 2 uses
        # only bank (P*4): one [128, 256] region holds all 4 gate chunks.
        psum = [
            ctx.enter_context(nc.psum_tensor(f"ps{i}", [128, 512], dt.float32))
            for i in range(8)
        ]
        block = ctx.enter_context(nc.Block())

        NIDX = NT // 16  # idx columns per tile
        NLOAD = 6 * 16   # preload DMA sem total

        # =========== SYNC engine: weight loads, phase-1 stores, ===========
        # =========== phase-2 gx prefetch, final output            ===========
        @block.sync
        def _(sy):
            # preload constants (HWDGE, FIFO order)
            sy.dma_start(
                out=idx_sb.ap().rearrange("p (t c) -> p t c", t=TT),
                in_=idx_d.ap().rearrange("t p c -> p t c"),
            ).then_inc(wload, 16)
            sy.dma_start(out=wih_sb[:, :], in_=wih_d[:, :]).then_inc(wload, 16)
            sy.dma_start(out=whh_sb[:, :], in_=whh_d[:, :]).then_inc(wload, 16)
            sy.dma_start(out=ident_sb[:, :], in_=ident_d[:, :]).then_inc(wload, 16)
            sy.dma_start(out=gbias_sb[:, :], in_=gbias_d[:, :]).then_inc(wload, 16)
            sy.dma_start(out=slot_sb[:, :], in_=slot_d[:, :]).then_inc(wload, 16)

            # phase-1 stores
            for tau in range(TT):
                sy.wait_ge(cp_sem, 4 * tau + 4)
                sy.dma_start(
                    out=gx_d[:, TPT * tau : TPT * (tau + 1), :],
                    in_=stage[tau % 2].ap().rearrange("p (t e) -> p t e", t=TPT),
                ).then_inc(st_sem[tau % 2], 16)

            # phase-2 gx prefetch: first two, then rolling
            sy.dma_start(out=gxt[0][:, :], in_=gx_d[:, 0, :]).then_inc(gxd[0], 16)
            if nsteps > 1:
                sy.dma_start(out=gxt[1][:, :], in_=gx_d[:, 1, :]).then_inc(gxd[1], 16)
            for t in range(2, nsteps):
                sy.wait_ge(idm, t - 1)
                sy.dma_start(out=gxt[t % 2][:, :], in_=gx_d[:, t, :]).then_inc(gxd[t % 2], 16)

            # final outputs
            sy.wait_ge(dve_s, 1 + 4 * nsteps)
            sy.dma_start(out=out_d[0, :, :], in_=hout_sb[:, :]).then_inc(fin, 16)
            sy.dma_start(out=out_d[1, :, :], in_=c_sb[:, :]).then_inc(fin, 16)
            sy.wait_ge(fin, 32)

        # =========== GPSIMD: barrier, gathers, h broadcast ===========
        @block.gpsimd
        def _(gp):
            # cross-core barrier: protects remote-sem increments from
            # racing a peer's kernel-start semaphore init.
            gp.memset(bar_sb[:, :], 0.0).then_inc(bar_sem, 1)
            gp.wait_ge(bar_sem, 1)
            gp.dma_start(out=bar_in[:, :], in_=bar_sb[:, :]).then_inc(bardma_sem, 16)
            gp.wait_ge(bardma_sem, 16)
            gp.collective_compute(
                "AllReduce",
                mybir.AluOpType.add,
                ins=[bar_in.ap().opt()],
                outs=[bar_out.ap().opt()],
                replica_groups=[list(range(CORES))],
            ).then_inc(cc_sem, 1)

            # phase-1 embedding gathers (transposing: out[p, k, j] = emb[idx_j, 128k+p])
            gp.wait_ge(wload, NLOAD)  # constants loaded (incl. idx_sb, slot)
            slot_r = gp.alloc_register("slot_r")
            gp.reg_load(slot_r, slot_sb[0:1, 0:1])
            for tau in range(TT):
                if tau >= 2:
                    gp.wait_ge(mm1, 4 * (tau - 2) + 4)  # xt buffer free
                gp.dma_gather(
                    out_ap=xt[tau % 2][:, :, :],
                    in_ap=emb_d[:, :],
                    idxs_ap=idx_sb[:, NIDX * tau : NIDX * (tau + 1)],
                    num_idxs=NT,
                    num_idxs_reg=NT,
                    elem_size=EMB,
                    transpose=True,
                ).then_inc(g_sem[tau % 2], 16)

            # phase-2 h exchange: one 8-dest broadcast (self included) per
            # step; every sender writes its own physical-slot column range
            # on every receiver. The SWDGE broadcast ucode rejects
            # register-offset APs, so branch once on the per-core slot id
            # and emit the loop with a static offset in each body.
            if exchange == "remote":
                gp.wait_ge(cc_sem, 1)
                for q in range(CORES):
                    with gp.If_eq(slot_r, q * B):
                        for t in range(nsteps - 1):
                            po = (t + 1) % 2  # parity of buffer holding h(t)
                            gp.remote_dma_broadcast(
                                out_ap=hg[po][:, q * B : (q + 1) * B],
                                in_ap=hmine[po][:, :],
                                remote_sem=rsem[po],
                                local_sem=lsem[po],
                                rdests=[(0, d) for d in range(CORES)],
                            ).then_inc(prep_s, 1)
                            gp.wait_ge(prep_s, t + 1)
                            gp.wait_ge(dve_s, 1 + 4 * t + 4)  # h(t) written
                            gp.trigger_dma(count=1)

        # =========== TENSOR engine ===========
        @block.tensor
        def _(te):
            te.wait_ge(wload, NLOAD)
            # ---- phase 1 ----
            for tau in range(TT):
                te.wait_ge(g_sem[tau % 2], 16 * (tau // 2 + 1))
                wihv = wih_sb.ap().rearrange("p (k g) -> p k g", k=KC)
                for cb in range(NCHUNK):
                    pb = psum[(tau % 2) * 4 + cb]
                    if tau >= 2:
                        te.wait_ge(cp_sem, 4 * (tau - 2) + cb + 1)
                    if False:
                        pass
                    else:
                        for k in range(KC):
                            mm = te.matmul(
                                pb[:, :],
                                lhsT=wih_sb[:, G * k + 128 * cb : G * k + 128 * (cb + 1)],
                                rhs=xt[tau % 2][:, k, :],
                                start=(k == 0),
                                stop=(k == KC - 1),
                            )
                    mm.then_inc(mm1, 1)

            # ---- phase 2 ----
            for t in range(nsteps):
                P = t % 2
                # identity-matmul loads gx_t into psum (one per gate bank)
                te.wait_ge(gxd[t % 2], 16 * (t // 2 + 1))
                if t < 2:
                    te.wait_ge(cp_sem, 4 * TT)  # phase-1 copies fully drained
                else:
                    te.wait_ge(act_s, 5 * (t - 2) + 4)  # psum parity reuse
                for cb in range(NCHUNK):
                    mm = te.matmul(
                        psum[P * 4 + cb][:, 0:B],
                        lhsT=ident_sb[:, :],
                        rhs=gxt[P][:, B * cb : B * (cb + 1)],
                        start=True,
                        stop=(t == 0),
                    )
                    if cb == NCHUNK - 1:
                        mm.then_inc(idm, 1)
                if t >= 1:
                    if exchange == "remote" and wait_rsem:
                        te.wait_ge(rsem[t % 2], 16 * ((t + 1) // 2))
                    if fp8 >= 2:
                        whv = whh_sb.ap().rearrange("p (d g) -> p d g", d=KC)
                        hgv = hg[P].ap().rearrange("p (s b) -> p s b", s=CORES)
                        for cb in range(NCHUNK):
                            for j in range(CORES // 2):
                                mm = te.matmul(
                                    psum[P * 4 + cb][:, 0:B],
                                    lhsT=whv[:, 2 * j : 2 * j + 2, 128 * cb : 128 * (cb + 1)],
                                    rhs=hgv[:, 2 * j : 2 * j + 2, :],
                                    start=False,
                                    stop=(j == CORES // 2 - 1),
                                    perf_mode=DR,
                                )
                            mm.then_inc(mmr, 1)
                    else:
                        for cb in range(NCHUNK):
                            for d in range(CORES):
                                mm = te.matmul(
                                    psum[P * 4 + cb][:, 0:B],
                                    lhsT=whh_sb[:, G * d + 128 * cb : G * d + 128 * (cb + 1)],
                                    rhs=hg[P][:, B * d : B * (d + 1)],
                                    start=False,
                                    stop=(d == CORES - 1),
                                )
                            mm.then_inc(mmr, 1)

        # =========== SCALAR engine (ACT) ===========
        @block.scalar
        def _(sc):
            sc.wait_ge(wload, NLOAD)
            # ---- phase 1: psum -> stage (bf16 cast) ----
            for tau in range(TT):
                for cb in range(NCHUNK):
                    sc.wait_ge(mm1, 4 * tau + cb + 1)
                    if tau >= 2:
                        sc.wait_ge(st_sem[tau % 2], 16 * (tau // 2))  # stage free
                    src = psum[(tau % 2) * 4 + cb].ap().rearrange("p (t b) -> p t b", t=TPT)
                    dst = stage[tau % 2].ap().rearrange(
                        "p (t e b) -> p t e b", t=TPT, e=NCHUNK
                    )[:, :, cb, :]
                    sc.activation(dst, src, AF.Copy).then_inc(cp_sem, 1)

            # ---- phase 2 activations ----
            # chunk order: 0=g(tanh), 1=i, 2=f, 3=o (sigmoid); then tanh(c)
            for t in range(nsteps):
                P = t % 2
                for cb in range(NCHUNK):
                    if t == 0:
                        sc.wait_ge(idm, 1)
                    else:
                        sc.wait_ge(mmr, 4 * (t - 1) + cb + 1)
                    fn = AF.Tanh if cb == 0 else AF.Sigmoid
                    sc.activation(
                        sg[:, B * cb : B * (cb + 1)],
                        psum[P * 4 + cb][:, 0:B],
                        fn,
                        bias=gbias_sb[:, cb : cb + 1],
                        scale=(1.0 / 64.0 if fp8 >= 2 else 1.0),
                    ).then_inc(act_s, 1)
                sc.wait_ge(dve_s, 1 + 4 * t + 3)  # c updated
                sc.activation(thc_sb[:, :], c_sb[:, :], AF.Tanh).then_inc(act_s, 1)

        # =========== VECTOR engine (DVE) ===========
        @block.vector
        def _(ve):
            ve.memset(c_sb[:, :], 0.0).then_inc(dve_s, 1)
            for t in range(nsteps):
                Pn = (t + 1) % 2
                ve.wait_ge(act_s, 5 * t + 2)
                ve.tensor_mul(ig_sb[:, :], sg[:, B : 2 * B], sg[:, 0:B]).then_inc(dve_s, 1)
                ve.wait_ge(act_s, 5 * t + 3)
                # c_sb RAW from previous step's update (or the memset)
                ve.wait_ge(dve_s, max(1, 1 + 4 * (t - 1) + 3))
                ve.tensor_mul(fc_sb[:, :], sg[:, 2 * B : 3 * B], c_sb[:, :]).then_inc(dve_s, 1)
                ve.wait_ge(dve_s, 1 + 4 * t + 2)  # ig, fc writebacks landed
                ve.tensor_add(c_sb[:, :], ig_sb[:, :], fc_sb[:, :]).then_inc(dve_s, 1)
                ve.wait_ge(act_s, 5 * t + 5)
                if t == nsteps - 1:
                    ve.tensor_mul(hout_sb[:, :], sg[:, 3 * B : 4 * B], thc_sb[:, :]).then_inc(dve_s, 1)
                else:
                    if t >= 2 and exchange == "remote":
                        ve.wait_ge(lsem[Pn], 16 * (t // 2))  # hmine[Pn] sent
                    ve.tensor_mul(hmine[Pn][:, :], sg[:, 3 * B : 4 * B], thc_sb[:, :]).then_inc(dve_s, 1)

    nc.compile()
    return nc


# ---------------------------------------------------------------------------
# host-side input prep
# ---------------------------------------------------------------------------

def prepare_in_maps(source, emb, W_ih, W_hh, b_ih, b_hh, nsteps=S, fp8=0):
    import ml_dtypes as _mld

    F8 = mybir.dt.np(mybir.dt.float8e4)
    source = np.asarray(source)
    emb = np.asarray(emb, np.float32)
    W_ih = np.asarray(W_ih, np.float32)
    W_hh = np.asarray(W_hh, np.float32)
    b = np.asarray(b_ih, np.float32) + np.asarray(b_hh, np.float32)

    TT = B * nsteps // NT
    emb16 = emb.astype(BF16)
    ident = (64.0 * np.eye(128) if fp8 >= 2 else np.eye(128)).astype(BF16)

    # fp8 transposing gather interleaves byte pairs at u16 granularity:
    # xt[p, f, tok] = emb_row[2p + 256*(f//2) + (f%2)]
    CH = np.zeros([KC, 128], np.int64)
    for f in range(KC):
        CH[f] = 2 * np.arange(128) + 256 * (f // 2) + (f % 2)

    # indices, wrapped: idx[tau, p, s] = source[b, TPT*tau + t'] with
    # j = s*16 + (p % 16), t' = j // 64, b = j % 64
    idx = np.zeros([TT, 128, NT // 16], np.int16)
    j = np.arange(NT)
    tprime, bb = j // B, j % B
    for tau in range(TT):
        ids = source[bb, TPT * tau + tprime].astype(np.int16)  # [NT]
        wrapped = ids.reshape(NT // 16, 16).T  # [16, NT//16]
        idx[tau] = np.tile(wrapped, (8, 1))

    in_maps = []
    H = HID
    for j_core in range(CORES):
        rows = np.concatenate(
            [
                np.arange(CHUNK_TO_BLOCK[cb] * H + 128 * j_core,
                          CHUNK_TO_BLOCK[cb] * H + 128 * (j_core + 1))
                for cb in range(NCHUNK)
            ]
        )
        Wi = W_ih[rows]  # [512, 1024]
        Wh = W_hh[rows]
        bi = b[rows]  # [512]

        # w_ih[p, G*k + 128*cb + m] = Wi[128*cb + m, 128*k + p]
        wi4 = Wi.reshape(NCHUNK, 128, KC, 128)          # [cb, m, k, p]
        wih = np.transpose(wi4, (3, 2, 0, 1)).reshape(128, KC * G).astype(BF16)

        # w_hh with physical-slot chunk order: slot q holds the h slice of
        # logical core _M[q] (same permutation on every core).
        wh4 = (64.0 * Wh if fp8 >= 2 else Wh).reshape(NCHUNK, 128, KC, 128)
        wh4p = wh4[:, :, _M, :]
        whh = np.transpose(wh4p, (3, 2, 0, 1)).reshape(128, KC * G).astype(
            F8 if fp8 >= 2 else BF16
        )

        gbias = bi.reshape(NCHUNK, 128).T.copy().astype(np.float32)  # [128, 4]

        in_maps.append(
            {
                "emb16": emb16,
                "idx": idx,
                "w_ih": wih,
                "w_hh": whh,
                "ident": ident,
                "gbias": gbias,
                "slot": np.array([[_M[j_core] * B]], np.int32),
            }
        )
    return in_maps


_BUILD_CACHE = {}


FP8_MODE = 0


def _get_nc(nsteps=S, exchange="remote", fp8=None):
    if fp8 is None:
        fp8 = FP8_MODE
    key = (nsteps, exchange, fp8)
    if key not in _BUILD_CACHE:
        _BUILD_CACHE[key] = build(nsteps, exchange, fp8=fp8)
    return _BUILD_CACHE[key]


def kernel(source, emb, W_ih, W_hh, b_ih, b_hh, _trace=False):
    from concourse.bass_utils import run_bass_kernel_spmd

    nc = _get_nc()
    in_maps = prepare_in_maps(source, emb, W_ih, W_hh, b_ih, b_hh, fp8=FP8_MODE)
    res = run_bass_kernel_spmd(nc, in_maps, core_ids=list(range(CORES)), trace=_trace)
    outs = [res.results[i]["out"] for i in range(CORES)]  # each [2, 128, B]
    h = np.concatenate([o[0].T for o in outs], axis=1)  # [B, 8*128]
    c = np.concatenate([o[1].T for o in outs], axis=1)
    out = np.stack([h, c]).astype(np.float32)
    if _trace:
        return out, res
    return out


# ---------------------------------------------------------------------------
# dev: multi-core simulation on a reduced problem
# ---------------------------------------------------------------------------

def _simulate(nsteps=8, exchange="remote", check_with_hw=False, fp8=0):
    from concourse import bass_interp, libnrt

    # no /dev/neuron on the axon client: fake the driver's logical->physical
    # NC map with the standard trn2 XOR-4 die-flip table.
    fake_map = {(d, i): _M[i] for d in range(16) for i in range(8)}
    libnrt.get_trn2_nc_mapping = lambda: fake_map
    libnrt.nc_to_real_nc = lambda dev, i: fake_map[(dev, i)]
    bass_interp.nc_to_real_nc = libnrt.nc_to_real_nc
    bass_interp.pnc_id_to_device_and_real_nc_index = (
        lambda core_id: (core_id // 8, fake_map[(core_id // 8, core_id % 8)])
    )
    fake_rid = {d: d for d in range(16)}
    libnrt.get_device_id_to_routing_id_mapping = lambda: fake_rid
    bass_interp.get_device_id_to_routing_id_mapping = lambda: fake_rid

    rng = np.random.default_rng(0)
    source = rng.integers(0, VOCAB, (B, nsteps)).astype(np.int32)
    emb = rng.standard_normal((VOCAB, EMB), np.float32)
    W_ih = (rng.standard_normal((4 * HID, EMB), np.float32) / np.sqrt(EMB)).astype(np.float32)
    W_hh = (rng.standard_normal((4 * HID, HID), np.float32) / np.sqrt(HID)).astype(np.float32)
    b_ih = np.zeros(4 * HID, np.float32)
    b_hh = np.zeros(4 * HID, np.float32)

    nc = build(nsteps, exchange, fp8=fp8)
    in_maps = prepare_in_maps(source, emb, W_ih, W_hh, b_ih, b_hh, nsteps, fp8=fp8)

    sim = bass_interp.MultiCoreSim(nc, CORES)
    for i in range(CORES):
        for k, v in in_maps[i].items():
            sim.cores[i].tensor(k)[:] = v
    sim.simulate(check_with_hw=check_with_hw)

    outs = [
        np.array(sim.cores[i].mem_tensor("out")).reshape(2, 128, B)
        for i in range(CORES)
    ]
    h = np.concatenate([o[0].T for o in outs], axis=1)
    c = np.concatenate([o[1].T for o in outs], axis=1)
    actual = np.stack([h, c])

    # numpy reference
    X = emb[source]  # [B, S, E]
    hh = np.zeros((B, HID), np.float32)
    cc = np.zeros((B, HID), np.float32)
    for t in range(nsteps):
        gates = X[:, t, :] @ W_ih.T + hh @ W_hh.T + b_ih + b_hh
        i_, f_, g_, o_ = np.split(gates, 4, axis=-1)
        i_ = 1 / (1 + np.exp(-i_))
        f_ = 1 / (1 + np.exp(-f_))
        g_ = np.tanh(g_)
        o_ = 1 / (1 + np.exp(-o_))
        cc = f_ * cc + i_ * g_
        hh = o_ * np.tanh(cc)
    expected = np.stack([hh, cc])
    err = np.abs(actual - expected).max() / np.abs(expected).max()
    times = [sim.cores[i].time for i in range(CORES)]
    print(f"sim nsteps={nsteps} absmax_rel_err={err:.3e} sim_time_ns={max(times)}")
    return err


if __name__ == "__main__":
    ns = int(sys.argv[1]) if len(sys.argv) > 1 else 8
    ex = sys.argv[2] if len(sys.argv) > 2 else "remote"
    f8 = int(sys.argv[3]) if len(sys.argv) > 3 else 0
    _simulate(ns, ex, fp8=f8)


# revision 15
# speedup vs baseline: 5.0970x; 5.0970x over previous
"""LSTM encoder (B=64, S=512, E=H=1024) on 8 trn2 NeuronCores.

Strategy:
  - Tensor-parallel over the 4H gate dimension: each core owns 128 hidden
    channels (x4 gates = 512 gate rows), the full batch (64), and the full
    sequence.
  - Embedding gather via dma_gather(transpose=True) directly into X^T
    layout; gx = W_ih_local @ X^T per 512-token tile, drained by VectorE
    into one of four SBUF stage buffers that the recurrence's identity
    matmuls read directly (no DRAM roundtrip).
  - Recurrence: 512 sequential steps. Per step: gates.T = identity-
    matmul(gx_t) into PSUM banks 0-3 + sum_k W_hh_chunk @ h_chunk (PSUM
    accumulate), sigmoid/tanh on ScalarE, cell update on VectorE. The new
    local h slice (128 ch x 64 batch, bf16) goes out as ONE
    remote_dma_broadcast to all 8 cores (self included): every sender
    writes its own physical-slot column range on every receiver (static
    offset selected by an If-chain on a per-core register), keeping the
    program SPMD and the per-step SWDGE cost to one prep + one trigger.
  - Phase-1 OVERLAY: the per-tile W_ih matmuls are interleaved into the
    recurrence's PE idle windows (4 matmuls per step on PSUM banks 4-7,
    tile t//8+2 during step t), so the input projection costs no extra
    wall-clock. VectorE drains tile PSUM into the bf16 stage buffers.
  - h lives in [-1,1]: bf16 exchange; c stays fp32 on-core.

Self-contained: hardcodes all shapes; host-side prep is numpy only.
"""

import os
import sys

sys.path.insert(0, "/opt/trn_rl_repo")

import numpy as np
import ml_dtypes

import concourse.bass as bass
import concourse.bacc as bacc
import concourse.mybir as mybir
from concourse.bass import ds

BF16 = ml_dtypes.bfloat16
AF = mybir.ActivationFunctionType
dt = mybir.dt

# problem constants
VOCAB, EMB, HID = 32000, 1024, 1024
B = 64
S = 512
CORES = 8
KC = 8            # contraction chunks of 128
NCHUNK = 4        # gate chunks per core (order: g, i, f, o)
G = NCHUNK * 128  # 512 gate rows per core
NT = 512          # tokens per phase-1 tile
TPT = NT // B     # timesteps per phase-1 tile (8)
# pytorch gate blocks in W rows: i, f, g, o ; our chunk order: g, i, f, o
CHUNK_TO_BLOCK = [2, 0, 1, 3]

# Logical(replica) -> physical TPB mapping on trn2 (driver V0 table, the
# per-chip base offset cancels): upper-die pairs are swapped. Broadcast
# slot q on every receiver holds the h slice of logical core _M[q]
# (each sender writes its own physical slot everywhere); any bijection
# works since every core uses the same slot->slice convention.
_M = [0, 1, 2, 3, 6, 7, 4, 5]


def build(nc_steps=S, exchange="remote", nbcast=7, wait_rsem=True, fp8=0):
    """Emit the SPMD bass program (identical on all 8 cores).

    fp8=0: bf16 everywhere. fp8=2: W_hh/h in fp8e4 DoubleRow (weights
    host-scaled by 64, gate activations rescale by 1/64).
    """
    nsteps = nc_steps
    TT = B * nsteps // NT  # number of phase-1 token tiles
    assert B * nsteps % NT == 0
    NPRO = min(TT, 2)      # prologue tiles (before the step loop)
    DR = mybir.MatmulPerfMode.DoubleRow
    h_dt = dt.float8e4 if fp8 >= 2 else dt.bfloat16

    def ov_tile(t):
        """Overlay tile computed during step t (or None)."""
        k = t // 8 + 2
        return k if NPRO <= 2 <= TT and 2 <= k < TT else None

    nc = bacc.Bacc(None, target_bir_lowering=False)

    # ---- kernel I/O (per core) ----
    emb_d = nc.declare_dram_parameter("emb16", [VOCAB, EMB], dt.bfloat16, isOutput=False)
    idx_d = nc.declare_dram_parameter("idx", [TT, 128, NT // 16], dt.int16, isOutput=False)
    wih_d = nc.declare_dram_parameter("w_ih", [128, KC * G], dt.bfloat16, isOutput=False)
    whh_d = nc.declare_dram_parameter("w_hh", [128, KC * G], h_dt, isOutput=False)
    ident_d = nc.declare_dram_parameter("ident", [128, 128], dt.bfloat16, isOutput=False)
    gbias_d = nc.declare_dram_parameter("gbias", [128, NCHUNK], dt.float32, isOutput=False)
    slot_d = nc.declare_dram_parameter("slot", [1, 1], dt.int32, isOutput=False)
    out_d = nc.declare_dram_parameter("out", [2, 128, B], dt.float32, isOutput=True)

    # ---- DRAM scratch ----
    bar_in = nc.dram_tensor("bar_in", [128, 4], dt.float32)
    bar_out = nc.dram_tensor("bar_out", [128, 4], dt.float32, addr_space="Shared")

    # ---- semaphores ----
    cc_sem = nc.alloc_semaphore("cc_sem")
    bar_sem = nc.alloc_semaphore("bar_sem")
    bardma_sem = nc.alloc_semaphore("bardma_sem")
    wload = nc.alloc_semaphore("wload")
    g_sem = [nc.alloc_semaphore("g_sem0"), nc.alloc_semaphore("g_sem1")]
    mm1 = nc.alloc_semaphore("mm1")
    cp_sem = nc.alloc_semaphore("cp_sem")
    idm = nc.alloc_semaphore("idm")
    mmr = nc.alloc_semaphore("mmr")
    act_s = nc.alloc_semaphore("act_s")
    dve_s = nc.alloc_semaphore("dve_s")
    prep_s = nc.alloc_semaphore("prep_s")
    # parity-split: exchange for h(t) uses index (t+1)%2; the 2-step
    # pipeline separation keeps parity chains disjoint.
    rsem = [nc.alloc_semaphore("rsem0"), nc.alloc_semaphore("rsem1")]  # +16 per exchange
    lsem = [nc.alloc_semaphore("lsem0"), nc.alloc_semaphore("lsem1")]  # +16 per exchange
    fin = nc.alloc_semaphore("fin")

    from contextlib import ExitStack

    with ExitStack() as ctx:
        sb = lambda name, shape, d: ctx.enter_context(nc.sbuf_tensor(name, shape, d))
        idx_sb = sb("idx_sb", [128, TT * (NT // 16)], dt.int16)
        wih_sb = sb("wih_sb", [128, KC * G], dt.bfloat16)
        whh_sb = sb("whh_sb", [128, KC * G], h_dt)
        ident_sb = sb("ident_sb", [128, 128], dt.bfloat16)
        gbias_sb = sb("gbias_sb", [128, NCHUNK], dt.float32)
        slot_sb = sb("slot_sb", [1, 1], dt.int32)
        xt = [sb(f"xt{i}", [128, KC, NT], dt.bfloat16) for i in range(2)]
        NSTG = 4
        stage = [sb(f"stage{i}", [128, TPT * NCHUNK * B], dt.bfloat16) for i in range(NSTG)]
        hg = [sb(f"hg{i}", [128, CORES * B], h_dt) for i in range(2)]
        hmine = [sb(f"hmine{i}", [128, B], h_dt) for i in range(2)]
        sg = sb("sg", [128, NCHUNK * B], dt.float32)
        ig_sb = sb("ig_sb", [128, B], dt.float32)
        fc_sb = sb("fc_sb", [128, B], dt.float32)
        thc_sb = sb("thc_sb", [128, B], dt.float32)
        c_sb = sb("c_sb", [128, B], dt.float32)
        hout_sb = sb("hout_sb", [128, B], dt.float32)
        bar_sb = sb("bar_sb", [128, 4], dt.float32)
        # PSUM: recurrence gates on banks 0-3 (one per gate chunk, single
        # set), overlay tile accumulation on banks 4-7.
        psum = [
            ctx.enter_context(nc.psum_tensor(f"ps{i}", [128, 512], dt.float32))
            for i in range(8)
        ]
        block = ctx.enter_context(nc.Block())

        NIDX = NT // 16  # idx columns per tile
        NLOAD = 6 * 16   # preload DMA sem total

        wihv = None  # set inside tensor block

        def p1_chain(te, k, cb, kks):
            """Emit W_ih matmuls (k-chunks kks) of tile k, gate chunk cb."""
            pb = psum[4 + cb]
            for kk in kks:
                if kk == 0 and k >= 1:
                    te.wait_ge(cp_sem, 4 * (k - 1) + cb + 1)  # bank drained
                mm = te.matmul(
                    pb[:, :],
                    lhsT=wih_sb[:, G * kk + 128 * cb : G * kk + 128 * (cb + 1)],
                    rhs=xt[k % 2][:, kk, :],
                    start=(kk == 0),
                    stop=(kk == KC - 1),
                )
                if kk == KC - 1:
                    mm.then_inc(mm1, 1)

        # =========== SYNC engine: loads, stores, gx prefetch, output ======
        @block.sync
        def _(sy):
            sy.dma_start(
                out=idx_sb.ap().rearrange("p (t c) -> p t c", t=TT),
                in_=idx_d.ap().rearrange("t p c -> p t c"),
            ).then_inc(wload, 16)
            sy.dma_start(out=wih_sb[:, :], in_=wih_d[:, :]).then_inc(wload, 16)
            sy.dma_start(out=whh_sb[:, :], in_=whh_d[:, :]).then_inc(wload, 16)
            sy.dma_start(out=ident_sb[:, :], in_=ident_d[:, :]).then_inc(wload, 16)
            sy.dma_start(out=gbias_sb[:, :], in_=gbias_d[:, :]).then_inc(wload, 16)
            sy.dma_start(out=slot_sb[:, :], in_=slot_d[:, :]).then_inc(wload, 16)

            sy.wait_ge(dve_s, 1 + 4 * nsteps)
            sy.dma_start(out=out_d[0, :, :], in_=hout_sb[:, :]).then_inc(fin, 16)
            sy.dma_start(out=out_d[1, :, :], in_=c_sb[:, :]).then_inc(fin, 16)
            sy.wait_ge(fin, 32)

        # =========== GPSIMD: barrier, gathers, h broadcast ===========
        @block.gpsimd
        def _(gp):
            # cross-core barrier: protects remote-sem increments from
            # racing a peer's kernel-start semaphore init.
            gp.memset(bar_sb[:, :], 0.0).then_inc(bar_sem, 1)
            gp.wait_ge(bar_sem, 1)
            gp.dma_start(out=bar_in[:, :], in_=bar_sb[:, :]).then_inc(bardma_sem, 16)
            gp.wait_ge(bardma_sem, 16)
            gp.collective_compute(
                "AllReduce",
                mybir.AluOpType.add,
                ins=[bar_in.ap().opt()],
                outs=[bar_out.ap().opt()],
                replica_groups=[list(range(CORES))],
            ).then_inc(cc_sem, 1)

            gp.wait_ge(wload, NLOAD)
            slot_r = gp.alloc_register("slot_r")
            gp.reg_load(slot_r, slot_sb[0:1, 0:1])

            def gather(k):
                if k >= 2:
                    gp.wait_ge(mm1, 4 * (k - 2) + 4)  # xt buffer free
                gp.dma_gather(
                    out_ap=xt[k % 2][:, :, :],
                    in_ap=emb_d[:, :],
                    idxs_ap=idx_sb[:, NIDX * k : NIDX * (k + 1)],
                    num_idxs=NT,
                    num_idxs_reg=NT,
                    elem_size=EMB,
                    transpose=True,
                ).then_inc(g_sem[k % 2], 16)

            for k in range(NPRO + 1 if TT > NPRO else NPRO):
                gather(k)  # prologue tiles + overlay tile 2 (used at t=0)

            # h exchange: one 8-dest broadcast (self included) per step;
            # the sender's physical slot is a static offset selected once
            # via an If-chain on the per-core slot register (the broadcast
            # ucode rejects register-offset APs). Later tile gathers ride
            # inside the same loop.
            if exchange == "remote":
                gp.wait_ge(cc_sem, 1)
                for q in range(CORES):
                    with gp.If_eq(slot_r, q * B):
                        for t in range(nsteps - 1):
                            kg = t // 8 + 3  # gather one tile ahead of use
                            if t % 8 == 0 and NPRO + 1 <= kg < TT:
                                gather(kg)
                            po = (t + 1) % 2  # parity of buffer holding h(t)
                            gp.remote_dma_broadcast(
                                out_ap=hg[po][:, q * B : (q + 1) * B],
                                in_ap=hmine[po][:, :],
                                remote_sem=rsem[po],
                                local_sem=lsem[po],
                                rdests=[(0, d) for d in range(CORES)],
                            ).then_inc(prep_s, 1)
                            gp.wait_ge(prep_s, t + 1)
                            gp.wait_ge(dve_s, 1 + 4 * t + 4)  # h(t) written
                            gp.trigger_dma(count=1)
            else:
                for k in range(NPRO + 1 if TT > NPRO else NPRO, TT):
                    gather(k)

        # =========== TENSOR engine ===========
        @block.tensor
        def _(te):
            te.wait_ge(wload, NLOAD)
            # prologue tiles
            for k in range(NPRO):
                te.wait_ge(g_sem[k % 2], 16 * (k // 2 + 1))
                for cb in range(NCHUNK):
                    p1_chain(te, k, cb, range(KC))

            # recurrence with overlay; gx_t is read straight out of the
            # stage buffer of tile t//8 (no DRAM roundtrip).
            for t in range(nsteps):
                kt, tp = t // 8, t % 8
                if tp == 0:
                    te.wait_ge(cp_sem, 4 * kt + 4)  # tile kt fully staged
                gxs = stage[kt % NSTG].ap().rearrange(
                    "p (u e b) -> p u e b", u=TPT, e=NCHUNK
                )
                for cb in range(NCHUNK):
                    if t >= 1:
                        te.wait_ge(act_s, 5 * (t - 1) + cb + 1)  # bank read
                    mm = te.matmul(
                        psum[cb][:, 0:B],
                        lhsT=ident_sb[:, :],
                        rhs=gxs[:, tp, cb, :],
                        start=True,
                        stop=(t == 0),
                    )
                    if cb == NCHUNK - 1:
                        mm.then_inc(idm, 1)
                if t >= 1:
                    if exchange == "remote" and wait_rsem:
                        te.wait_ge(rsem[t % 2], 16 * ((t + 1) // 2))
                    if fp8 >= 2:
                        whv = whh_sb.ap().rearrange("p (d g) -> p d g", d=KC)
                        hgv = hg[t % 2].ap().rearrange("p (s b) -> p s b", s=CORES)
                        for cb in range(NCHUNK):
                            for j in range(CORES // 2):
                                mm = te.matmul(
                                    psum[cb][:, 0:B],
                                    lhsT=whv[:, 2 * j : 2 * j + 2, 128 * cb : 128 * (cb + 1)],
                                    rhs=hgv[:, 2 * j : 2 * j + 2, :],
                                    start=False,
                                    stop=(j == CORES // 2 - 1),
                                    perf_mode=DR,
                                )
                            mm.then_inc(mmr, 1)
                    else:
                        for cb in range(NCHUNK):
                            for d in range(CORES):
                                mm = te.matmul(
                                    psum[cb][:, 0:B],
                                    lhsT=whh_sb[:, G * d + 128 * cb : G * d + 128 * (cb + 1)],
                                    rhs=hg[t % 2][:, B * d : B * (d + 1)],
                                    start=False,
                                    stop=(d == CORES - 1),
                                )
                            mm.then_inc(mmr, 1)
                # overlay quanta: 4 W_ih matmuls of tile t//8+2
                kov = ov_tile(t)
                if kov is not None:
                    j0 = 4 * (t % 8)
                    if j0 == 0:
                        te.wait_ge(g_sem[kov % 2], 16 * (kov // 2 + 1))
                    for j in range(j0, j0 + 4):
                        p1_chain(te, kov, j // 8, [j % 8])

        # =========== SCALAR engine (ACT): gate activations only ===========
        @block.scalar
        def _(sc):
            sc.wait_ge(wload, NLOAD)
            # chunk order: 0=g(tanh), 1=i, 2=f, 3=o (sigmoid); then tanh(c)
            for t in range(nsteps):
                for cb in range(NCHUNK):
                    if t == 0:
                        sc.wait_ge(idm, 1)
                    else:
                        sc.wait_ge(mmr, 4 * (t - 1) + cb + 1)
                    fn = AF.Tanh if cb == 0 else AF.Sigmoid
                    sc.activation(
                        sg[:, B * cb : B * (cb + 1)],
                        psum[cb][:, 0:B],
                        fn,
                        bias=gbias_sb[:, cb : cb + 1],
                        scale=(1.0 / 64.0 if fp8 >= 2 else 1.0),
                    ).then_inc(act_s, 1)
                sc.wait_ge(dve_s, 1 + 4 * t + 3)  # c updated
                sc.activation(thc_sb[:, :], c_sb[:, :], AF.Tanh).then_inc(act_s, 1)

        # =========== VECTOR engine (DVE): cell update + stage copies ======
        @block.vector
        def _(ve):
            def copy_chunk(k, cb):
                ve.wait_ge(mm1, 4 * k + cb + 1)
                if k >= NSTG:
                    ve.wait_ge(idm, 8 * (k - NSTG) + 8)  # stage slot free
                src = psum[4 + cb].ap().rearrange("p (t b) -> p t b", t=TPT)
                dst = stage[k % NSTG].ap().rearrange(
                    "p (t e b) -> p t e b", t=TPT, e=NCHUNK
                )[:, :, cb, :]
                ve.tensor_copy(dst, src).then_inc(cp_sem, 1)

            ve.memset(c_sb[:, :], 0.0).then_inc(dve_s, 1)
            for k in range(NPRO):
                for cb in range(NCHUNK):
                    copy_chunk(k, cb)

            for t in range(nsteps):
                Pn = (t + 1) % 2
                ve.wait_ge(act_s, 5 * t + 2)
                ve.tensor_mul(ig_sb[:, :], sg[:, B : 2 * B], sg[:, 0:B]).then_inc(dve_s, 1)
                ve.wait_ge(act_s, 5 * t + 3)
                # c_sb RAW from previous step's update (or the memset)
                ve.wait_ge(dve_s, max(1, 1 + 4 * (t - 1) + 3))
                ve.tensor_mul(fc_sb[:, :], sg[:, 2 * B : 3 * B], c_sb[:, :]).then_inc(dve_s, 1)
                ve.wait_ge(dve_s, 1 + 4 * t + 2)  # ig, fc writebacks landed
                ve.tensor_add(c_sb[:, :], ig_sb[:, :], fc_sb[:, :]).then_inc(dve_s, 1)
                ve.wait_ge(act_s, 5 * t + 5)
                if t == nsteps - 1:
                    ve.tensor_mul(hout_sb[:, :], sg[:, 3 * B : 4 * B], thc_sb[:, :]).then_inc(dve_s, 1)
                else:
                    if t >= 2 and exchange == "remote":
                        ve.wait_ge(lsem[Pn], 16 * (t // 2))  # hmine[Pn] sent
                    ve.tensor_mul(hmine[Pn][:, :], sg[:, 3 * B : 4 * B], thc_sb[:, :]).then_inc(dve_s, 1)
                # drain a finished overlay chain into the stage buffer
                kov = ov_tile(t)
                if kov is not None and t % 8 in (1, 3, 5, 7):
                    copy_chunk(kov, (t % 8 - 1) // 2)

    nc.compile()
    return nc


# ---------------------------------------------------------------------------
# host-side input prep
# ---------------------------------------------------------------------------

def prepare_in_maps(source, emb, W_ih, W_hh, b_ih, b_hh, nsteps=S, fp8=0):
    F8 = mybir.dt.np(mybir.dt.float8e4)
    source = np.asarray(source)
    emb = np.asarray(emb, np.float32)
    W_ih = np.asarray(W_ih, np.float32)
    W_hh = np.asarray(W_hh, np.float32)
    b = np.asarray(b_ih, np.float32) + np.asarray(b_hh, np.float32)

    TT = B * nsteps // NT
    emb16 = emb.astype(BF16)
    ident = ((64.0 if fp8 >= 2 else 1.0) * np.eye(128)).astype(BF16)

    # indices, wrapped: idx[tau, p, s] = source[b, TPT*tau + t'] with
    # j = s*16 + (p % 16), t' = j // 64, b = j % 64
    idx = np.zeros([TT, 128, NT // 16], np.int16)
    j = np.arange(NT)
    tprime, bb = j // B, j % B
    for tau in range(TT):
        ids = source[bb, TPT * tau + tprime].astype(np.int16)  # [NT]
        wrapped = ids.reshape(NT // 16, 16).T  # [16, NT//16]
        idx[tau] = np.tile(wrapped, (8, 1))

    in_maps = []
    H = HID
    for j_core in range(CORES):
        rows = np.concatenate(
            [
                np.arange(CHUNK_TO_BLOCK[cb] * H + 128 * j_core,
                          CHUNK_TO_BLOCK[cb] * H + 128 * (j_core + 1))
                for cb in range(NCHUNK)
            ]
        )
        Wi = W_ih[rows]  # [512, 1024]
        Wh = W_hh[rows]
        bi = b[rows]  # [512]

        # w_ih[p, G*k + 128*cb + m] = Wi[128*cb + m, 128*k + p]
        wi4 = Wi.reshape(NCHUNK, 128, KC, 128)          # [cb, m, k, p]
        wih = np.transpose(wi4, (3, 2, 0, 1)).reshape(128, KC * G).astype(BF16)

        # w_hh with physical-slot chunk order: slot q holds the h slice of
        # logical core _M[q] (same permutation on every core).
        wh4 = (64.0 * Wh if fp8 >= 2 else Wh).reshape(NCHUNK, 128, KC, 128)
        wh4p = wh4[:, :, _M, :]
        whh = np.transpose(wh4p, (3, 2, 0, 1)).reshape(128, KC * G).astype(
            F8 if fp8 >= 2 else BF16
        )

        gbias = bi.reshape(NCHUNK, 128).T.copy().astype(np.float32)  # [128, 4]

        in_maps.append(
            {
                "emb16": emb16,
                "idx": idx,
                "w_ih": wih,
                "w_hh": whh,
                "ident": ident,
                "gbias": gbias,
                "slot": np.array([[_M[j_core] * B]], np.int32),
            }
        )
    return in_maps


_BUILD_CACHE = {}

FP8_MODE = 0


def _get_nc(nsteps=S, exchange="remote", fp8=None):
    if fp8 is None:
        fp8 = FP8_MODE
    key = (nsteps, exchange, fp8)
    if key not in _BUILD_CACHE:
        _BUILD_CACHE[key] = build(nsteps, exchange, fp8=fp8)
    return _BUILD_CACHE[key]


def kernel(source, emb, W_ih, W_hh, b_ih, b_hh, _trace=False):
    from concourse.bass_utils import run_bass_kernel_spmd

    nc = _get_nc()
    in_maps = prepare_in_maps(source, emb, W_ih, W_hh, b_ih, b_hh, fp8=FP8_MODE)
    res = run_bass_kernel_spmd(nc, in_maps, core_ids=list(range(CORES)), trace=_trace)
    outs = [res.results[i]["out"] for i in range(CORES)]  # each [2, 128, B]
    h = np.concatenate([o[0].T for o in outs], axis=1)  # [B, 8*128]
    c = np.concatenate([o[1].T for o in outs], axis=1)
    out = np.stack([h, c]).astype(np.float32)
    if _trace:
        return out, res
    return out


# ---------------------------------------------------------------------------
# dev: multi-core simulation on a reduced problem
# ---------------------------------------------------------------------------

def _simulate(nsteps=8, exchange="remote", check_with_hw=False, fp8=0):
    from concourse import bass_interp, libnrt

    # no /dev/neuron on the axon client: fake the driver's logical->physical
    # NC map with the standard trn2 XOR-4 die-flip table.
    fake_map = {(d, i): _M[i] for d in range(16) for i in range(8)}
    libnrt.get_trn2_nc_mapping = lambda: fake_map
    libnrt.nc_to_real_nc = lambda dev, i: fake_map[(dev, i)]
    bass_interp.nc_to_real_nc = libnrt.nc_to_real_nc
    bass_interp.pnc_id_to_device_and_real_nc_index = (
        lambda core_id: (core_id // 8, fake_map[(core_id // 8, core_id % 8)])
    )
    fake_rid = {d: d for d in range(16)}
    libnrt.get_device_id_to_routing_id_mapping = lambda: fake_rid
    bass_interp.get_device_id_to_routing_id_mapping = lambda: fake_rid

    rng = np.random.default_rng(0)
    source = rng.integers(0, VOCAB, (B, nsteps)).astype(np.int32)
    emb = rng.standard_normal((VOCAB, EMB), np.float32)
    W_ih = (rng.standard_normal((4 * HID, EMB), np.float32) / np.sqrt(EMB)).astype(np.float32)
    W_hh = (rng.standard_normal((4 * HID, HID), np.float32) / np.sqrt(HID)).astype(np.float32)
    b_ih = np.zeros(4 * HID, np.float32)
    b_hh = np.zeros(4 * HID, np.float32)

    nc = build(nsteps, exchange, fp8=fp8)
    in_maps = prepare_in_maps(source, emb, W_ih, W_hh, b_ih, b_hh, nsteps, fp8=fp8)

    sim = bass_interp.MultiCoreSim(nc, CORES)
    for i in range(CORES):
        for k, v in in_maps[i].items():
            sim.cores[i].tensor(k)[:] = v
    sim.simulate(check_with_hw=check_with_hw)

    outs = [
        np.array(sim.cores[i].mem_tensor("out")).reshape(2, 128, B)
        for i in range(CORES)
    ]
    h = np.concatenate([o[0].T for o in outs], axis=1)
    c = np.concatenate([o[1].T for o in outs], axis=1)
    actual = np.stack([h, c])

    # numpy reference
    X = emb[source]  # [B, S, E]
    hh = np.zeros((B, HID), np.float32)
    cc = np.zeros((B, HID), np.float32)
    for t in range(nsteps):
        gates = X[:, t, :] @ W_ih.T + hh @ W_hh.T + b_ih + b_hh
        i_, f_, g_, o_ = np.split(gates, 4, axis=-1)
        i_ = 1 / (1 + np.exp(-i_))
        f_ = 1 / (1 + np.exp(-f_))
        g_ = np.tanh(g_)
        o_ = 1 / (1 + np.exp(-o_))
        cc = f_ * cc + i_ * g_
        hh = o_ * np.tanh(cc)
    expected = np.stack([hh, cc])
    err = np.abs(actual - expected).max() / np.abs(expected).max()
    times = [sim.cores[i].time for i in range(CORES)]
    print(f"sim nsteps={nsteps} absmax_rel_err={err:.3e} sim_time_ns={max(times)}")
    return err


if __name__ == "__main__":
    ns = int(sys.argv[1]) if len(sys.argv) > 1 else 8
    ex = sys.argv[2] if len(sys.argv) > 2 else "remote"
    f8 = int(sys.argv[3]) if len(sys.argv) > 3 else 0
    _simulate(ns, ex, fp8=f8)


# revision 17
# speedup vs baseline: 20.1494x; 3.9532x over previous
"""LSTM encoder (B=64, S=512, E=H=1024) on 8 trn2 NeuronCores.

Strategy:
  - Tensor-parallel over the 4H gate dimension: each core owns 128 hidden
    channels (x4 gates = 512 gate rows), the full batch (64), and the full
    sequence.
  - Phase 1 (parallel): embedding gather via dma_gather(transpose=True)
    directly into X^T layout, then gx = W_ih_local @ X^T for all 32768
    tokens, stored to DRAM as bf16.
  - Phase 2 (recurrence): 512 sequential steps. Per step: gates.T =
    identity-matmul(gx_t) + sum_k W_hh_chunk @ h_chunk (PSUM accumulate,
    one shared 256-col bank region per parity), sigmoid/tanh on ScalarE,
    cell update on VectorE. The new local h slice (128 ch x 64 batch,
    bf16) goes out as ONE remote_dma_broadcast to all 8 cores (self
    included) with a register-valued destination offset: every sender
    writes its own physical-slot column range on every receiver, so the
    SPMD program stays identical across cores and the per-step SWDGE
    desc-gen cost is one prep + one trigger instead of seven.
  - h lives in [-1,1]: bf16 exchange; c stays fp32 on-core.

Self-contained: hardcodes all shapes; host-side prep is numpy only.
"""

import os
import sys

sys.path.insert(0, "/opt/trn_rl_repo")

import numpy as np
import ml_dtypes

import concourse.bass as bass
import concourse.bacc as bacc
import concourse.mybir as mybir
from concourse.bass import ds

BF16 = ml_dtypes.bfloat16
AF = mybir.ActivationFunctionType
dt = mybir.dt

# problem constants
VOCAB, EMB, HID = 32000, 1024, 1024
B = 64
S = 512
CORES = 8
KC = 8            # contraction chunks of 128
NCHUNK = 4        # gate chunks per core (order: g, i, f, o)
G = NCHUNK * 128  # 512 gate rows per core
NT = 512          # tokens per phase-1 tile
TPT = NT // B     # timesteps per phase-1 tile (8)
# pytorch gate blocks in W rows: i, f, g, o ; our chunk order: g, i, f, o
CHUNK_TO_BLOCK = [2, 0, 1, 3]

# Logical(replica) -> physical TPB mapping on trn2 (driver V0 table, the
# per-chip base offset cancels): upper-die pairs are swapped. Broadcast
# slot q on every receiver holds the h slice of logical core _M[q]
# (each sender writes its own physical slot everywhere). _M is an
# involution, so _M[_M[q]] = q. HW-verified by a probe kernel.
_M = [0, 1, 2, 3, 6, 7, 4, 5]


def build(nc_steps=S, exchange="remote", nbcast=7, wait_rsem=True, fp8=0):
    """Emit the SPMD bass program (identical on all 8 cores).

    fp8=0: bf16 everywhere. fp8=1: phase-1 W_ih/x in fp8e4 (DoubleRow).
    fp8=2: also phase-2 W_hh/h in fp8e4. Weights are host-scaled by 64;
    activations rescale by 1/64.
    """
    nsteps = nc_steps
    TT = B * nsteps // NT  # number of phase-1 token tiles
    assert B * nsteps % NT == 0
    DR = mybir.MatmulPerfMode.DoubleRow
    x_dt = dt.bfloat16
    h_dt = dt.float8e4 if fp8 >= 2 else dt.bfloat16

    nc = bacc.Bacc(None, target_bir_lowering=False)

    # ---- kernel I/O (per core) ----
    emb_d = nc.declare_dram_parameter("emb16", [VOCAB, EMB], x_dt, isOutput=False)
    idx_d = nc.declare_dram_parameter("idx", [TT, 128, NT // 16], dt.int16, isOutput=False)
    wih_d = nc.declare_dram_parameter("w_ih", [128, KC * G], dt.bfloat16, isOutput=False)
    whh_d = nc.declare_dram_parameter("w_hh", [128, KC * G], h_dt, isOutput=False)
    ident_d = nc.declare_dram_parameter("ident", [128, 128], dt.bfloat16, isOutput=False)
    gbias_d = nc.declare_dram_parameter("gbias", [128, NCHUNK], dt.float32, isOutput=False)
    slot_d = nc.declare_dram_parameter("slot", [1, 1], dt.int32, isOutput=False)
    out_d = nc.declare_dram_parameter("out", [2, 128, B], dt.float32, isOutput=True)

    # ---- DRAM scratch ----
    gx_d = nc.dram_tensor("gx", [128, nsteps, NCHUNK * B], dt.bfloat16)
    bar_in = nc.dram_tensor("bar_in", [128, 4], dt.float32)
    bar_out = nc.dram_tensor("bar_out", [128, 4], dt.float32, addr_space="Shared")

    # ---- semaphores ----
    cc_sem = nc.alloc_semaphore("cc_sem")
    bar_sem = nc.alloc_semaphore("bar_sem")
    bardma_sem = nc.alloc_semaphore("bardma_sem")
    wload = nc.alloc_semaphore("wload")
    g_sem = [nc.alloc_semaphore("g_sem0"), nc.alloc_semaphore("g_sem1")]
    mm1 = nc.alloc_semaphore("mm1")
    cp_sem = nc.alloc_semaphore("cp_sem")
    st_sem = [nc.alloc_semaphore("st_sem0"), nc.alloc_semaphore("st_sem1")]
    gxd = [nc.alloc_semaphore("gxd0"), nc.alloc_semaphore("gxd1")]
    idm = nc.alloc_semaphore("idm")
    mmr = nc.alloc_semaphore("mmr")
    act_s = nc.alloc_semaphore("act_s")
    dve_s = nc.alloc_semaphore("dve_s")
    prep_s = nc.alloc_semaphore("prep_s")
    # parity-split: exchange for h(t) uses index (t+1)%2; the 2-step
    # pipeline separation keeps parity chains disjoint.
    rsem = [nc.alloc_semaphore("rsem0"), nc.alloc_semaphore("rsem1")]  # +16 per exchange
    lsem = [nc.alloc_semaphore("lsem0"), nc.alloc_semaphore("lsem1")]  # +16 per exchange
    fin = nc.alloc_semaphore("fin")

    from contextlib import ExitStack

    with ExitStack() as ctx:
        sb = lambda name, shape, d: ctx.enter_context(nc.sbuf_tensor(name, shape, d))
        idx_sb = sb("idx_sb", [128, TT * (NT // 16)], dt.int16)
        wih_sb = sb("wih_sb", [128, KC * G], dt.bfloat16)
        whh_sb = sb("whh_sb", [128, KC * G], h_dt)
        ident_sb = sb("ident_sb", [128, 128], dt.bfloat16)
        gbias_sb = sb("gbias_sb", [128, NCHUNK], dt.float32)
        slot_sb = sb("slot_sb", [1, 1], dt.int32)
        xt = [sb(f"xt{i}", [128, KC, NT], x_dt) for i in range(2)]
        stage = [sb(f"stage{i}", [128, TPT * NCHUNK * B], dt.bfloat16) for i in range(2)]
        hg = [sb(f"hg{i}", [128, CORES * B], h_dt) for i in range(2)]
        hmine = [sb(f"hmine{i}", [128, B], h_dt) for i in range(2)]
        gxt = [sb(f"gxt{i}", [128, NCHUNK * B], dt.bfloat16) for i in range(2)]
        sg = sb("sg", [128, NCHUNK * B], dt.float32)
        ig_sb = sb("ig_sb", [128, B], dt.float32)
        fc_sb = sb("fc_sb", [128, B], dt.float32)
        thc_sb = sb("thc_sb", [128, B], dt.float32)
        c_sb = sb("c_sb", [128, B], dt.float32)
        hout_sb = sb("hout_sb", [128, B], dt.float32)
        bar_sb = sb("bar_sb", [128, 4], dt.float32)
        # PSUM: 8 tensors of [128, 512] fp32 = 8 full banks. Phase 2 uses
        # only bank (P*4): one [128, 256] region holds all 4 gate chunks.
        psum = [
            ctx.enter_context(nc.psum_tensor(f"ps{i}", [128, 512], dt.float32))
            for i in range(8)
        ]
        block = ctx.enter_context(nc.Block())

        NIDX = NT // 16  # idx columns per tile
        NLOAD = 6 * 16   # preload DMA sem total

        # =========== SYNC engine: weight loads, phase-1 stores, ===========
        # =========== phase-2 gx prefetch, final output            ===========
        @block.sync
        def _(sy):
            # preload constants (HWDGE, FIFO order)
            sy.dma_start(
                out=idx_sb.ap().rearrange("p (t c) -> p t c", t=TT),
                in_=idx_d.ap().rearrange("t p c -> p t c"),
            ).then_inc(wload, 16)
            sy.dma_start(out=wih_sb[:, :], in_=wih_d[:, :]).then_inc(wload, 16)
            sy.dma_start(out=whh_sb[:, :], in_=whh_d[:, :]).then_inc(wload, 16)
            sy.dma_start(out=ident_sb[:, :], in_=ident_d[:, :]).then_inc(wload, 16)
            sy.dma_start(out=gbias_sb[:, :], in_=gbias_d[:, :]).then_inc(wload, 16)
            sy.dma_start(out=slot_sb[:, :], in_=slot_d[:, :]).then_inc(wload, 16)

            # phase-1 stores
            for tau in range(TT):
                sy.wait_ge(cp_sem, 4 * tau + 4)
                sy.dma_start(
                    out=gx_d[:, TPT * tau : TPT * (tau + 1), :],
                    in_=stage[tau % 2].ap().rearrange("p (t e) -> p t e", t=TPT),
                ).then_inc(st_sem[tau % 2], 16)

            # phase-2 gx prefetch: first two, then rolling
            sy.dma_start(out=gxt[0][:, :], in_=gx_d[:, 0, :]).then_inc(gxd[0], 16)
            if nsteps > 1:
                sy.dma_start(out=gxt[1][:, :], in_=gx_d[:, 1, :]).then_inc(gxd[1], 16)
            for t in range(2, nsteps):
                sy.wait_ge(idm, t - 1)
                sy.dma_start(out=gxt[t % 2][:, :], in_=gx_d[:, t, :]).then_inc(gxd[t % 2], 16)

            # final outputs
            sy.wait_ge(dve_s, 1 + 4 * nsteps)
            sy.dma_start(out=out_d[0, :, :], in_=hout_sb[:, :]).then_inc(fin, 16)
            sy.dma_start(out=out_d[1, :, :], in_=c_sb[:, :]).then_inc(fin, 16)
            sy.wait_ge(fin, 32)

        # =========== GPSIMD: barrier, gathers, h broadcast ===========
        @block.gpsimd
        def _(gp):
            # cross-core barrier: protects remote-sem increments from
            # racing a peer's kernel-start semaphore init.
            gp.memset(bar_sb[:, :], 0.0).then_inc(bar_sem, 1)
            gp.wait_ge(bar_sem, 1)
            gp.dma_start(out=bar_in[:, :], in_=bar_sb[:, :]).then_inc(bardma_sem, 16)
            gp.wait_ge(bardma_sem, 16)
            gp.collective_compute(
                "AllReduce",
                mybir.AluOpType.add,
                ins=[bar_in.ap().opt()],
                outs=[bar_out.ap().opt()],
                replica_groups=[list(range(CORES))],
            ).then_inc(cc_sem, 1)

            # phase-1 embedding gathers (transposing: out[p, k, j] = emb[idx_j, 128k+p])
            gp.wait_ge(wload, NLOAD)  # constants loaded (incl. idx_sb, slot)
            slot_r = gp.alloc_register("slot_r")
            gp.reg_load(slot_r, slot_sb[0:1, 0:1])
            for tau in range(TT):
                if tau >= 2:
                    gp.wait_ge(mm1, 4 * (tau - 2) + 4)  # xt buffer free
                gp.dma_gather(
                    out_ap=xt[tau % 2][:, :, :],
                    in_ap=emb_d[:, :],
                    idxs_ap=idx_sb[:, NIDX * tau : NIDX * (tau + 1)],
                    num_idxs=NT,
                    num_idxs_reg=NT,
                    elem_size=EMB,
                    transpose=True,
                ).then_inc(g_sem[tau % 2], 16)

            # phase-2 h exchange: one 8-dest broadcast (self included) per
            # step; every sender writes its own physical-slot column range
            # on every receiver. The SWDGE broadcast ucode rejects
            # register-offset APs, so branch once on the per-core slot id
            # and emit the loop with a static offset in each body.
            if exchange == "remote":
                gp.wait_ge(cc_sem, 1)
                for q in range(CORES):
                    with gp.If_eq(slot_r, q * B):
                        for t in range(nsteps - 1):
                            po = (t + 1) % 2  # parity of buffer holding h(t)
                            gp.remote_dma_broadcast(
                                out_ap=hg[po][:, q * B : (q + 1) * B],
                                in_ap=hmine[po][:, :],
                                remote_sem=rsem[po],
                                local_sem=lsem[po],
                                rdests=[(0, d) for d in range(CORES)],
                            ).then_inc(prep_s, 1)
                            gp.wait_ge(prep_s, t + 1)
                            gp.wait_ge(dve_s, 1 + 4 * t + 4)  # h(t) written
                            gp.trigger_dma(count=1)

        # =========== TENSOR engine ===========
        @block.tensor
        def _(te):
            te.wait_ge(wload, NLOAD)
            # ---- phase 1 ----
            for tau in range(TT):
                te.wait_ge(g_sem[tau % 2], 16 * (tau // 2 + 1))
                wihv = wih_sb.ap().rearrange("p (k g) -> p k g", k=KC)
                for cb in range(NCHUNK):
                    pb = psum[(tau % 2) * 4 + cb]
                    if tau >= 2:
                        te.wait_ge(cp_sem, 4 * (tau - 2) + cb + 1)
                    if False:
                        pass
                    else:
                        for k in range(KC):
                            mm = te.matmul(
                                pb[:, :],
                                lhsT=wih_sb[:, G * k + 128 * cb : G * k + 128 * (cb + 1)],
                                rhs=xt[tau % 2][:, k, :],
                                start=(k == 0),
                                stop=(k == KC - 1),
                            )
                    mm.then_inc(mm1, 1)

            # ---- phase 2 ----
            for t in range(nsteps):
                P = t % 2
                # identity-matmul loads gx_t into psum (one per gate bank)
                te.wait_ge(gxd[t % 2], 16 * (t // 2 + 1))
                if t < 2:
                    te.wait_ge(cp_sem, 4 * TT)  # phase-1 copies fully drained
                else:
                    te.wait_ge(act_s, 5 * (t - 2) + 4)  # psum parity reuse
                for cb in range(NCHUNK):
                    mm = te.matmul(
                        psum[P * 4 + cb][:, 0:B],
                        lhsT=ident_sb[:, :],
                        rhs=gxt[P][:, B * cb : B * (cb + 1)],
                        start=True,
                        stop=(t == 0),
                    )
                    if cb == NCHUNK - 1:
                        mm.then_inc(idm, 1)
                if t >= 1:
                    if exchange == "remote" and wait_rsem:
                        te.wait_ge(rsem[t % 2], 16 * ((t + 1) // 2))
                    if fp8 >= 2:
                        whv = whh_sb.ap().rearrange("p (d g) -> p d g", d=KC)
                        hgv = hg[P].ap().rearrange("p (s b) -> p s b", s=CORES)
                        for cb in range(NCHUNK):
                            for j in range(CORES // 2):
                                mm = te.matmul(
                                    psum[P * 4 + cb][:, 0:B],
                                    lhsT=whv[:, 2 * j : 2 * j + 2, 128 * cb : 128 * (cb + 1)],
                                    rhs=hgv[:, 2 * j : 2 * j + 2, :],
                                    start=False,
                                    stop=(j == CORES // 2 - 1),
                                    perf_mode=DR,
                                )
                            mm.then_inc(mmr, 1)
                    else:
                        for cb in range(NCHUNK):
                            for d in range(CORES):
                                mm = te.matmul(
                                    psum[P * 4 + cb][:, 0:B],
                                    lhsT=whh_sb[:, G * d + 128 * cb : G * d + 128 * (cb + 1)],
                                    rhs=hg[P][:, B * d : B * (d + 1)],
                                    start=False,
                                    stop=(d == CORES - 1),
                                )
                            mm.then_inc(mmr, 1)

        # =========== SCALAR engine (ACT) ===========
        @block.scalar
        def _(sc):
            sc.wait_ge(wload, NLOAD)
            # ---- phase 1: psum -> stage (bf16 cast) ----
            for tau in range(TT):
                for cb in range(NCHUNK):
                    sc.wait_ge(mm1, 4 * tau + cb + 1)
                    if tau >= 2:
                        sc.wait_ge(st_sem[tau % 2], 16 * (tau // 2))  # stage free
                    src = psum[(tau % 2) * 4 + cb].ap().rearrange("p (t b) -> p t b", t=TPT)
                    dst = stage[tau % 2].ap().rearrange(
                        "p (t e b) -> p t e b", t=TPT, e=NCHUNK
                    )[:, :, cb, :]
                    sc.activation(dst, src, AF.Copy).then_inc(cp_sem, 1)

            # ---- phase 2 activations ----
            # chunk order: 0=g(tanh), 1=i, 2=f, 3=o (sigmoid); then tanh(c)
            for t in range(nsteps):
                P = t % 2
                for cb in range(NCHUNK):
                    if t == 0:
                        sc.wait_ge(idm, 1)
                    else:
                        sc.wait_ge(mmr, 4 * (t - 1) + cb + 1)
                    fn = AF.Tanh if cb == 0 else AF.Sigmoid
                    sc.activation(
                        sg[:, B * cb : B * (cb + 1)],
                        psum[P * 4 + cb][:, 0:B],
                        fn,
                        bias=gbias_sb[:, cb : cb + 1],
                        scale=(1.0 / 64.0 if fp8 >= 2 else 1.0),
                    ).then_inc(act_s, 1)
                sc.wait_ge(dve_s, 1 + 4 * t + 3)  # c updated
                sc.activation(thc_sb[:, :], c_sb[:, :], AF.Tanh).then_inc(act_s, 1)

        # =========== VECTOR engine (DVE) ===========
        @block.vector
        def _(ve):
            ve.memset(c_sb[:, :], 0.0).then_inc(dve_s, 1)
            for t in range(nsteps):
                Pn = (t + 1) % 2
                ve.wait_ge(act_s, 5 * t + 2)
                ve.tensor_mul(ig_sb[:, :], sg[:, B : 2 * B], sg[:, 0:B]).then_inc(dve_s, 1)
                ve.wait_ge(act_s, 5 * t + 3)
                # c_sb RAW from previous step's update (or the memset)
                ve.wait_ge(dve_s, max(1, 1 + 4 * (t - 1) + 3))
                ve.tensor_mul(fc_sb[:, :], sg[:, 2 * B : 3 * B], c_sb[:, :]).then_inc(dve_s, 1)
                ve.wait_ge(dve_s, 1 + 4 * t + 2)  # ig, fc writebacks landed
                ve.tensor_add(c_sb[:, :], ig_sb[:, :], fc_sb[:, :]).then_inc(dve_s, 1)
                ve.wait_ge(act_s, 5 * t + 5)
                if t == nsteps - 1:
                    ve.tensor_mul(hout_sb[:, :], sg[:, 3 * B : 4 * B], thc_sb[:, :]).then_inc(dve_s, 1)
                else:
                    if t >= 2 and exchange == "remote":
                        ve.wait_ge(lsem[Pn], 16 * (t // 2))  # hmine[Pn] sent
                    ve.tensor_mul(hmine[Pn][:, :], sg[:, 3 * B : 4 * B], thc_sb[:, :]).then_inc(dve_s, 1)

    nc.compile()
    return nc


# ---------------------------------------------------------------------------
# host-side input prep
# ---------------------------------------------------------------------------

def prepare_in_maps(source, emb, W_ih, W_hh, b_ih, b_hh, nsteps=S, fp8=0):
    import ml_dtypes as _mld

    F8 = mybir.dt.np(mybir.dt.float8e4)
    source = np.asarray(source)
    emb = np.asarray(emb, np.float32)
    W_ih = np.asarray(W_ih, np.float32)
    W_hh = np.asarray(W_hh, np.float32)
    b = np.asarray(b_ih, np.float32) + np.asarray(b_hh, np.float32)

    TT = B * nsteps // NT
    emb16 = emb.astype(BF16)
    ident = (64.0 * np.eye(128) if fp8 >= 2 else np.eye(128)).astype(BF16)

    # fp8 transposing gather interleaves byte pairs at u16 granularity:
    # xt[p, f, tok] = emb_row[2p + 256*(f//2) + (f%2)]
    CH = np.zeros([KC, 128], np.int64)
    for f in range(KC):
        CH[f] = 2 * np.arange(128) + 256 * (f // 2) + (f % 2)

    # indices, wrapped: idx[tau, p, s] = source[b, TPT*tau + t'] with
    # j = s*16 + (p % 16), t' = j // 64, b = j % 64
    idx = np.zeros([TT, 128, NT // 16], np.int16)
    j = np.arange(NT)
    tprime, bb = j // B, j % B
    for tau in range(TT):
        ids = source[bb, TPT * tau + tprime].astype(np.int16)  # [NT]
        wrapped = ids.reshape(NT // 16, 16).T  # [16, NT//16]
        idx[tau] = np.tile(wrapped, (8, 1))

    in_maps = []
    H = HID
    for j_core in range(CORES):
        rows = np.concatenate(
            [
                np.arange(CHUNK_TO_BLOCK[cb] * H + 128 * j_core,
                          CHUNK_TO_BLOCK[cb] * H + 128 * (j_core + 1))
                for cb in range(NCHUNK)
            ]
        )
        Wi = W_ih[rows]  # [512, 1024]
        Wh = W_hh[rows]
        bi = b[rows]  # [512]

        # w_ih[p, G*k + 128*cb + m] = Wi[128*cb + m, 128*k + p]
        wi4 = Wi.reshape(NCHUNK, 128, KC, 128)          # [cb, m, k, p]
        wih = np.transpose(wi4, (3, 2, 0, 1)).reshape(128, KC * G).astype(BF16)

        # w_hh with physical-slot chunk order: slot q holds the h slice of
        # logical core _M[q] (same permutation on every core).
        wh4 = (64.0 * Wh if fp8 >= 2 else Wh).reshape(NCHUNK, 128, KC, 128)
        wh4p = wh4[:, :, _M, :]
        whh = np.transpose(wh4p, (3, 2, 0, 1)).reshape(128, KC * G).astype(
            F8 if fp8 >= 2 else BF16
        )

        gbias = bi.reshape(NCHUNK, 128).T.copy().astype(np.float32)  # [128, 4]

        in_maps.append(
            {
                "emb16": emb16,
                "idx": idx,
                "w_ih": wih,
                "w_hh": whh,
                "ident": ident,
                "gbias": gbias,
                "slot": np.array([[_M[j_core] * B]], np.int32),
            }
        )
    return in_maps


_BUILD_CACHE = {}


FP8_MODE = 0


def _get_nc(nsteps=S, exchange="remote", fp8=None):
    if fp8 is None:
        fp8 = FP8_MODE
    key = (nsteps, exchange, fp8)
    if key not in _BUILD_CACHE:
        _BUILD_CACHE[key] = build(nsteps, exchange, fp8=fp8)
    return _BUILD_CACHE[key]


def kernel(source, emb, W_ih, W_hh, b_ih, b_hh, _trace=False):
    from concourse.bass_utils import run_bass_kernel_spmd

    nc = _get_nc()
    in_maps = prepare_in_maps(source, emb, W_ih, W_hh, b_ih, b_hh, fp8=FP8_MODE)
    res = run_bass_kernel_spmd(nc, in_maps, core_ids=list(range(CORES)), trace=_trace)
    outs = [res.results[i]["out"] for i in range(CORES)]  # each [2, 128, B]
    h = np.concatenate([o[0].T for o in outs], axis=1)  # [B, 8*128]
    c = np.concatenate([o[1].T for o in outs], axis=1)
    out = np.stack([h, c]).astype(np.float32)
    if _trace:
        return out, res
    return out


# ---------------------------------------------------------------------------
# dev: multi-core simulation on a reduced problem
# ---------------------------------------------------------------------------

def _simulate(nsteps=8, exchange="remote", check_with_hw=False, fp8=0):
    from concourse import bass_interp, libnrt

    # no /dev/neuron on the axon client: fake the driver's logical->physical
    # NC map with the standard trn2 XOR-4 die-flip table.
    fake_map = {(d, i): _M[i] for d in range(16) for i in range(8)}
    libnrt.get_trn2_nc_mapping = lambda: fake_map
    libnrt.nc_to_real_nc = lambda dev, i: fake_map[(dev, i)]
    bass_interp.nc_to_real_nc = libnrt.nc_to_real_nc
    bass_interp.pnc_id_to_device_and_real_nc_index = (
        lambda core_id: (core_id // 8, fake_map[(core_id // 8, core_id % 8)])
    )
    fake_rid = {d: d for d in range(16)}
    libnrt.get_device_id_to_routing_id_mapping = lambda: fake_rid
    bass_interp.get_device_id_to_routing_id_mapping = lambda: fake_rid

    rng = np.random.default_rng(0)
    source = rng.integers(0, VOCAB, (B, nsteps)).astype(np.int32)
    emb = rng.standard_normal((VOCAB, EMB), np.float32)
    W_ih = (rng.standard_normal((4 * HID, EMB), np.float32) / np.sqrt(EMB)).astype(np.float32)
    W_hh = (rng.standard_normal((4 * HID, HID), np.float32) / np.sqrt(HID)).astype(np.float32)
    b_ih = np.zeros(4 * HID, np.float32)
    b_hh = np.zeros(4 * HID, np.float32)

    nc = build(nsteps, exchange, fp8=fp8)
    in_maps = prepare_in_maps(source, emb, W_ih, W_hh, b_ih, b_hh, nsteps, fp8=fp8)

    sim = bass_interp.MultiCoreSim(nc, CORES)
    for i in range(CORES):
        for k, v in in_maps[i].items():
            sim.cores[i].tensor(k)[:] = v
    sim.simulate(check_with_hw=check_with_hw)

    outs = [
        np.array(sim.cores[i].mem_tensor("out")).reshape(2, 128, B)
        for i in range(CORES)
    ]
    h = np.concatenate([o[0].T for o in outs], axis=1)
    c = np.concatenate([o[1].T for o in outs], axis=1)
    actual = np.stack([h, c])

    # numpy reference
    X = emb[source]  # [B, S, E]
    hh = np.zeros((B, HID), np.float32)
    cc = np.zeros((B, HID), np.float32)
    for t in range(nsteps):
        gates = X[:, t, :] @ W_ih.T + hh @ W_hh.T + b_ih + b_hh
        i_, f_, g_, o_ = np.split(gates, 4, axis=-1)
        i_ = 1 / (1 + np.exp(-i_))
        f_ = 1 / (1 + np.exp(-f_))
        g_ = np.tanh(g_)
        o_ = 1 / (1 + np.exp(-o_))
        cc = f_ * cc + i_ * g_
        hh = o_ * np.tanh(cc)
    expected = np.stack([hh, cc])
    err = np.abs(actual - expected).max() / np.abs(expected).max()
    times = [sim.cores[i].time for i in range(CORES)]
    print(f"sim nsteps={nsteps} absmax_rel_err={err:.3e} sim_time_ns={max(times)}")
    return err


if __name__ == "__main__":
    ns = int(sys.argv[1]) if len(sys.argv) > 1 else 8
    ex = sys.argv[2] if len(sys.argv) > 2 else "remote"
    f8 = int(sys.argv[3]) if len(sys.argv) > 3 else 0
    _simulate(ns, ex, fp8=f8)
